# revision 5
# baseline (speedup 1.0000x reference)
"""IrrepsLinear Trainium2 kernel: y = per-irrep-block x @ W / sqrt(mul).

Irreps layout: 256x0e + 128x1o + 64x2e -> blocks of width 256*1, 128*3, 64*5.
Data-parallel over 8 NeuronCores: each core gets 12500 nodes.

Strategy (v3):
  - fp16 DRAM IO halves HBM traffic (the roofline-binding resource);
    matmuls run fp16 x fp16 -> fp32 PSUM, evac casts back to fp16.
  - Host pre-permutes features so each 128-row K-group is one contiguous
    chunk in DRAM ([NW, 7, 128, SW] layout): every DMA is a plain
    contiguous 640 KB chunk transfer, streamed per-group so loads,
    compute, evac and stores pipeline at chunk granularity (no bursty
    window-end stores, tiny ramp-in/out exposure).
  - Block2's five 64-wide m-components: pairs (m0,m1), (m2,m3) via a
    128x128 block-diagonal W2 stationary (full PE width); m4 plain.
  - Block0 accumulates K=256 in a 5-bank [128, SW] PSUM tile with
    back-to-back same-stationary matmul runs (minimizes PE dead time),
    single wide evac. B1/B2 rotate 3 one-bank PSUM tiles.
"""

import numpy as np

NCORES = 8
N_TOTAL = 100000
NSH = N_TOTAL // NCORES   # 12500 nodes per core
NW = 5                    # windows per core
SW = NSH // NW            # 2500 node columns per window
D = 960
MMW = 512                 # matmul slice width (= one fp32 PSUM bank)

DT_IO = "float16"
_BUILD_CACHE = {}


def _perm():
    p = list(range(256))
    for m in range(3):
        p += [256 + 3 * i + m for i in range(128)]
    for m in range(5):
        p += [640 + 5 * i + m for i in range(64)]
    return np.asarray(p, dtype=np.int64)

_PERM = _perm()


def _build_program():
    import concourse.bass as bass  # noqa: F401
    import concourse.bacc as bacc
    import concourse.mybir as mybir
    import concourse.tile as tile

    key = (DT_IO, SW, MMW, NW, "v3")
    if key in _BUILD_CACHE:
        return _BUILD_CACHE[key]

    dt = getattr(mybir.dt, DT_IO)
    f32 = mybir.dt.float32

    nc = bacc.Bacc(
        "TRN2", target_bir_lowering=False, debug=False, enable_asserts=False
    )
    xa = nc.dram_tensor("xa", [NW, 7, 128, SW], dt, kind="ExternalInput").ap()
    xb = nc.dram_tensor("xb", [NW, 64, SW], dt, kind="ExternalInput").ap()
    w0 = nc.dram_tensor("w0", [256, 256], dt, kind="ExternalInput").ap()
    w1 = nc.dram_tensor("w1", [128, 128], dt, kind="ExternalInput").ap()
    w2d = nc.dram_tensor("w2d", [128, 128], dt, kind="ExternalInput").ap()
    w2s = nc.dram_tensor("w2s", [64, 64], dt, kind="ExternalInput").ap()
    ya = nc.dram_tensor("ya", [NW, 7, 128, SW], dt, kind="ExternalOutput").ap()
    yb = nc.dram_tensor("yb", [NW, 64, SW], dt, kind="ExternalOutput").ap()

    slices = [
        (i * MMW, min((i + 1) * MMW, SW)) for i in range((SW + MMW - 1) // MMW)
    ]

    with tile.TileContext(nc) as tc:
        with (
            tc.tile_pool(name="const", bufs=1) as cpool,
            tc.tile_pool(name="xc", bufs=10) as xcp,
            tc.tile_pool(name="xbp", bufs=2) as xbp,
            tc.tile_pool(name="yc", bufs=6) as ycp,
            tc.tile_pool(name="ybp", bufs=2) as ybp,
            tc.tile_pool(name="psf", bufs=1, space="PSUM") as psf,
            tc.tile_pool(name="psx", bufs=3, space="PSUM") as psx,
        ):
            # --- stationary weights, loaded once ---
            w0t0 = cpool.tile([128, 256], dt, name="w0t0", tag="w0t0")
            nc.sync.dma_start(w0t0[:], w0[0:128, :])
            w0t1 = cpool.tile([128, 256], dt, name="w0t1", tag="w0t1")
            nc.sync.dma_start(w0t1[:], w0[128:256, :])
            w1t = cpool.tile([128, 128], dt, name="w1t", tag="w1t")
            nc.sync.dma_start(w1t[:], w1[:, :])
            w2dt = cpool.tile([128, 128], dt, name="w2dt", tag="w2dt")
            nc.sync.dma_start(w2dt[:], w2d[:, :])
            w2st = cpool.tile([64, 64], dt, name="w2st", tag="w2st")
            nc.sync.dma_start(w2st[:], w2s[:, :])

            n_evac = 0

            def evac_small(dst, src):
                nonlocal n_evac
                n_evac += 1
                if n_evac % 2:
                    nc.scalar.copy(dst, src)
                else:
                    nc.vector.tensor_copy(dst, src)

            for w in range(NW):
                # chunk loads, in consumption order (SP HWDGE ring)
                xct = []
                for t in range(7):
                    ct = xcp.tile([128, SW], dt, name=f"xc{t}_{w}", tag="xc")
                    nc.sync.dma_start(ct[:], xa[w, t])
                    xct.append(ct)
                xbt = xbp.tile([64, SW], dt, name=f"xb_{w}", tag="xb")
                nc.sync.dma_start(xbt[:], xb[w])

                def b0_group(ob):
                    # K=256 accumulated in one 5-bank PSUM tile; two
                    # same-stationary matmul runs; one wide evac (DVE).
                    oc = slice(128 * ob, 128 * (ob + 1))
                    ps = psf.tile([128, SW], f32, name=f"psf{ob}_{w}", tag="psf")
                    for lo, hi in slices:
                        nc.tensor.matmul(
                            ps[:, lo:hi], w0t0[:, oc], xct[0][:, lo:hi],
                            start=True, stop=False,
                        )
                    for lo, hi in slices:
                        nc.tensor.matmul(
                            ps[:, lo:hi], w0t1[:, oc], xct[1][:, lo:hi],
                            start=False, stop=True,
                        )
                    yt = ycp.tile([128, SW], dt, name=f"yc{ob}_{w}", tag="yc")
                    nc.vector.tensor_copy(yt[:], ps[:, :SW])
                    nc.scalar.dma_start(ya[w, ob], yt[:])

                def small_group(t, stat, xt, prows=128):
                    # one-bank PSUM rotation; same stationary across slices
                    yt = ycp.tile([128, SW], dt, name=f"yc{t}_{w}", tag="yc") \
                        if prows == 128 else \
                        ybp.tile([64, SW], dt, name=f"yb_{w}", tag="yb")
                    for lo, hi in slices:
                        n = hi - lo
                        ps = psx.tile([128, MMW], f32, name=f"px{t}_{w}_{lo}",
                                      tag="psx")
                        nc.tensor.matmul(
                            ps[0:prows, :n], stat, xt[:, lo:hi],
                            start=True, stop=True,
                        )
                        evac_small(yt[0:prows, lo:hi], ps[0:prows, :n])
                    if prows == 128:
                        nc.scalar.dma_start(ya[w, t], yt[:])
                    else:
                        nc.scalar.dma_start(yb[w], yt[:])

                b0_group(0)
                small_group(2, w1t[:], xct[2])          # B1 m0
                b0_group(1)
                small_group(3, w1t[:], xct[3])          # B1 m1
                small_group(4, w1t[:], xct[4])          # B1 m2
                small_group(5, w2dt[:], xct[5])         # B2 m0|m1
                small_group(6, w2dt[:], xct[6])         # B2 m2|m3
                small_group(7, w2st[:], xbt, prows=64)  # B2 m4

    nc.compile()
    _BUILD_CACHE[key] = nc
    return nc


TRACE = False
LAST_RESULT = None


def kernel(x, W0, W1, W2):
    from concourse import bass_utils

    nc = _build_program()

    npdt = np.float16 if DT_IO == "float16" else None
    if npdt is None:
        import ml_dtypes
        npdt = ml_dtypes.bfloat16

    w0s = (np.asarray(W0, np.float32) / np.sqrt(256.0)).astype(npdt)
    w1s = (np.asarray(W1, np.float32) / np.sqrt(128.0)).astype(npdt)
    w2 = (np.asarray(W2, np.float32) / np.sqrt(64.0)).astype(npdt)
    w2d = np.zeros((128, 128), dtype=npdt)
    w2d[0:64, 0:64] = w2
    w2d[64:128, 64:128] = w2

    # pack x: feature-permuted, chunked [core][w][t][p][n]
    xh = np.asarray(x)[:, _PERM].astype(npdt)
    A = xh.reshape(NCORES, NW, SW, D)
    xa_all = np.ascontiguousarray(
        A[:, :, :, :896].reshape(NCORES, NW, SW, 7, 128).transpose(0, 1, 3, 4, 2)
    )
    xb_all = np.ascontiguousarray(A[:, :, :, 896:].transpose(0, 1, 3, 2))

    in_maps = []
    for c in range(NCORES):
        in_maps.append({
            "xa": xa_all[c], "xb": xb_all[c],
            "w0": w0s, "w1": w1s, "w2d": w2d, "w2s": w2,
        })

    res = bass_utils.run_bass_kernel_spmd(
        nc, in_maps, core_ids=list(range(NCORES)), trace=TRACE
    )
    global LAST_RESULT
    LAST_RESULT = res

    out = np.empty((N_TOTAL, D), dtype=np.float32)
    Yp = np.empty((NCORES, NW, SW, D), dtype=np.float32)
    for c in range(NCORES):
        ya = res.results[c]["ya"]    # [NW, 7, 128, SW]
        yb = res.results[c]["yb"]    # [NW, 64, SW]
        Yp[c, :, :, :896] = (
            ya.transpose(0, 3, 1, 2).reshape(NW, SW, 896)
        )
        Yp[c, :, :, 896:] = yb.transpose(0, 2, 1)
    out[:, _PERM] = Yp.reshape(N_TOTAL, D)
    return out


# revision 6
# speedup vs baseline: 1.1242x; 1.1242x over previous
"""IrrepsLinear Trainium2 kernel: y = per-irrep-block x @ W / sqrt(mul).

Irreps layout: 256x0e + 128x1o + 64x2e -> blocks of width 256*1, 128*3, 64*5.
Data-parallel over 8 NeuronCores: each core gets 12500 nodes.

Strategy (v4 = v2 + ramp smoothing):
  - fp16 DRAM IO halves HBM traffic (the roofline-binding resource);
    matmuls run fp16 x fp16 -> fp32 PSUM, evac casts back to fp16.
  - Host pre-permutes features so each 128-row K-group is contiguous and
    pre-packs node windows into contiguous DRAM blocks: one big load +
    one big store per window (monolithic transfers keep DMA at peak rate).
  - Ragged window sizes (small at both ends, 2500 in the middle) shrink
    the unoverlapped first-load / last-store pipeline exposure.
  - Weight loads go on the ACT HWDGE ring so the SP ring's first dispatch
    is the first x window.
  - Block2 m-pairs via a 128x128 block-diagonal W2 stationary; m4 plain.
  - 8 one-bank PSUM tiles rotate per 512-slice; evac alternates DVE/ACT.
"""

import numpy as np

NCORES = 8
N_TOTAL = 100000
NSH = N_TOTAL // NCORES   # 12500 nodes per core
D = 960
MMW = 512                 # matmul slice width (= one fp32 PSUM bank)

# ragged windows: small ends for pipeline ramp, big middle for DMA rate
WINDOWS = [512, 1024, 2048, 2500, 2500, 2000, 1024, 512, 380]
assert sum(WINDOWS) == NSH
OFFS = np.concatenate([[0], np.cumsum(WINDOWS)[:-1]]).tolist()

DT_IO = "float16"
_BUILD_CACHE = {}


def _perm():
    p = list(range(256))
    for m in range(3):
        p += [256 + 3 * i + m for i in range(128)]
    for m in range(5):
        p += [640 + 5 * i + m for i in range(64)]
    return np.asarray(p, dtype=np.int64)

_PERM = _perm()


def _build_program():
    import concourse.bass as bass  # noqa: F401
    import concourse.bacc as bacc
    import concourse.mybir as mybir
    import concourse.tile as tile

    key = (DT_IO, MMW, tuple(WINDOWS), "v4")
    if key in _BUILD_CACHE:
        return _BUILD_CACHE[key]

    dt = getattr(mybir.dt, DT_IO)
    f32 = mybir.dt.float32

    nc = bacc.Bacc(
        "TRN2", target_bir_lowering=False, debug=False, enable_asserts=False
    )
    xa = nc.dram_tensor("xa", [128, 7 * NSH], dt, kind="ExternalInput").ap()
    xbd = nc.dram_tensor("xb", [64, NSH], dt, kind="ExternalInput").ap()
    w0 = nc.dram_tensor("w0", [256, 256], dt, kind="ExternalInput").ap()
    w1 = nc.dram_tensor("w1", [128, 128], dt, kind="ExternalInput").ap()
    w2d = nc.dram_tensor("w2d", [128, 128], dt, kind="ExternalInput").ap()
    w2s = nc.dram_tensor("w2s", [64, 64], dt, kind="ExternalInput").ap()
    ya = nc.dram_tensor("ya", [128, 7 * NSH], dt, kind="ExternalOutput").ap()
    ybd = nc.dram_tensor("yb", [64, NSH], dt, kind="ExternalOutput").ap()

    with tile.TileContext(nc) as tc:
        with (
            tc.tile_pool(name="const", bufs=1) as cpool,
            tc.tile_pool(name="xin", bufs=2) as xpool,
            tc.tile_pool(name="xbp", bufs=2) as xbp,
            tc.tile_pool(name="yst", bufs=2) as ypool,
            tc.tile_pool(name="ybp", bufs=2) as ybp,
            tc.tile_pool(name="ps", bufs=8, space="PSUM") as pspool,
        ):
            # weights on the ACT HWDGE ring: SP's first dispatch is x data
            w0t0 = cpool.tile([128, 256], dt, name="w0t0", tag="w0t0")
            nc.scalar.dma_start(w0t0[:], w0[0:128, :])
            w0t1 = cpool.tile([128, 256], dt, name="w0t1", tag="w0t1")
            nc.scalar.dma_start(w0t1[:], w0[128:256, :])
            w1t = cpool.tile([128, 128], dt, name="w1t", tag="w1t")
            nc.scalar.dma_start(w1t[:], w1[:, :])
            w2dt = cpool.tile([128, 128], dt, name="w2dt", tag="w2dt")
            nc.scalar.dma_start(w2dt[:], w2d[:, :])
            w2st = cpool.tile([64, 64], dt, name="w2st", tag="w2st")
            nc.scalar.dma_start(w2st[:], w2s[:, :])

            n_evac = 0

            def evac(dst, src):
                nonlocal n_evac
                n_evac += 1
                if n_evac % 2:
                    nc.vector.tensor_copy(dst, src)
                else:
                    nc.scalar.copy(dst, src)

            for wi, (c0, sw) in enumerate(zip(OFFS, WINDOWS)):
                xat = xpool.tile([128, 7 * sw], dt, name=f"xa{wi}", tag="xa")
                nc.sync.dma_start(xat[:], xa[:, 7 * c0 : 7 * (c0 + sw)])
                xbt = xbp.tile([64, sw], dt, name=f"xb{wi}", tag="xb")
                nc.sync.dma_start(xbt[:], xbd[:, c0 : c0 + sw])
                yat = ypool.tile([128, 7 * sw], dt, name=f"ya{wi}", tag="ya")
                ybt = ybp.tile([64, sw], dt, name=f"yb{wi}", tag="yb")

                slices = [
                    (i * MMW, min((i + 1) * MMW, sw))
                    for i in range((sw + MMW - 1) // MMW)
                ]
                for lo, hi in slices:
                    n = hi - lo

                    def pst(nm):
                        return pspool.tile(
                            [128, MMW], f32, name=f"{nm}_{wi}_{lo}", tag="ps"
                        )

                    # block0: 256x0e (K=256 via 2 accum steps, M=256 via 2 obs)
                    for ob in range(2):
                        ps = pst(f"ps_b0_{ob}")
                        oc = slice(128 * ob, 128 * (ob + 1))
                        nc.tensor.matmul(
                            ps[:, :n], w0t0[:, oc], xat[:, 0 * sw + lo : 0 * sw + hi],
                            start=True, stop=False,
                        )
                        nc.tensor.matmul(
                            ps[:, :n], w0t1[:, oc], xat[:, 1 * sw + lo : 1 * sw + hi],
                            start=False, stop=True,
                        )
                        evac(yat[:, ob * sw + lo : ob * sw + hi], ps[:, :n])

                    # block1: 128x1o, 3 m-components
                    for m in range(3):
                        ps = pst(f"ps_b1_{m}")
                        t = 2 + m
                        nc.tensor.matmul(
                            ps[:, :n], w1t[:], xat[:, t * sw + lo : t * sw + hi],
                            start=True, stop=True,
                        )
                        evac(yat[:, t * sw + lo : t * sw + hi], ps[:, :n])

                    # block2: m-pairs via block-diag W2 (full PE width)
                    for g in range(2):
                        ps = pst(f"ps_b2_{g}")
                        t = 5 + g
                        nc.tensor.matmul(
                            ps[:, :n], w2dt[:], xat[:, t * sw + lo : t * sw + hi],
                            start=True, stop=True,
                        )
                        evac(yat[:, t * sw + lo : t * sw + hi], ps[:, :n])

                    # block2 m=4: plain 64-wide matmul
                    ps = pst("ps_b2_4")
                    nc.tensor.matmul(
                        ps[0:64, :n], w2st[:], xbt[:, lo:hi],
                        start=True, stop=True,
                    )
                    evac(ybt[:, lo:hi], ps[0:64, :n])

                # stores on ACT ring; in the last window, store yb first so
                # the final completion wait is on the wide 128-partition DMA
                if wi == len(WINDOWS) - 1:
                    nc.scalar.dma_start(ybd[:, c0 : c0 + sw], ybt[:])
                    nc.scalar.dma_start(ya[:, 7 * c0 : 7 * (c0 + sw)], yat[:])
                else:
                    nc.scalar.dma_start(ya[:, 7 * c0 : 7 * (c0 + sw)], yat[:])
                    nc.scalar.dma_start(ybd[:, c0 : c0 + sw], ybt[:])

    nc.compile()
    _BUILD_CACHE[key] = nc
    return nc


TRACE = False
LAST_RESULT = None


def kernel(x, W0, W1, W2):
    from concourse import bass_utils

    nc = _build_program()

    npdt = np.float16 if DT_IO == "float16" else None
    if npdt is None:
        import ml_dtypes
        npdt = ml_dtypes.bfloat16

    w0s = (np.asarray(W0, np.float32) / np.sqrt(256.0)).astype(npdt)
    w1s = (np.asarray(W1, np.float32) / np.sqrt(128.0)).astype(npdt)
    w2 = (np.asarray(W2, np.float32) / np.sqrt(64.0)).astype(npdt)
    w2d = np.zeros((128, 128), dtype=npdt)
    w2d[0:64, 0:64] = w2
    w2d[64:128, 64:128] = w2

    # pack x: feature-permuted; per ragged window: [128, t, n] blocks
    xh = np.asarray(x)[:, _PERM].astype(npdt)
    A = xh.reshape(NCORES, NSH, D)
    blocks = []
    for c0, sw in zip(OFFS, WINDOWS):
        blk = A[:, c0 : c0 + sw, :896].reshape(NCORES, sw, 7, 128)
        blocks.append(blk.transpose(0, 3, 2, 1).reshape(NCORES, 128, 7 * sw))
    xa_all = np.ascontiguousarray(np.concatenate(blocks, axis=2))
    xb_all = np.ascontiguousarray(A[:, :, 896:].transpose(0, 2, 1))

    in_maps = []
    for c in range(NCORES):
        in_maps.append({
            "xa": xa_all[c], "xb": xb_all[c],
            "w0": w0s, "w1": w1s, "w2d": w2d, "w2s": w2,
        })

    res = bass_utils.run_bass_kernel_spmd(
        nc, in_maps, core_ids=list(range(NCORES)), trace=TRACE
    )
    global LAST_RESULT
    LAST_RESULT = res

    out = np.empty((N_TOTAL, D), dtype=np.float32)
    Yp = np.empty((NCORES, NSH, D), dtype=np.float32)
    for c in range(NCORES):
        yac = res.results[c]["ya"]    # [128, 7*NSH]
        ybc = res.results[c]["yb"]    # [64, NSH]
        for c0, sw in zip(OFFS, WINDOWS):
            blk = yac[:, 7 * c0 : 7 * (c0 + sw)].reshape(128, 7, sw)
            Yp[c, c0 : c0 + sw, :896] = (
                blk.transpose(2, 1, 0).reshape(sw, 896)
            )
        Yp[c, :, 896:] = ybc.T
    out[:, _PERM] = Yp.reshape(N_TOTAL, D)
    return out
